# revision 17
# baseline (speedup 1.0000x reference)
"""CASSI GAP reconstruction (DifferentiableGAPTV) on 8 Trainium2 NeuronCores.

Strategy: shard H=512 rows across 8 cores as 128-row slabs (64 output rows +
32-row halo each side). Rows are independent except the 5x5 depthwise conv
(+-2 rows/iter * 12 iters = 24-row dependency), so the halo makes the whole
12-iteration loop collective-free; each core's central 64 rows are exact.

Per-core, fully SBUF-resident. Per iteration:
  A:  yb = sum_l shift_l(m*x_l)       -- DVE masked mults + PE fp32r identity
                                         matmuls accumulating a PSUM plane
  B:  y1 += y-yb; r = (y1-yb)/Phi     -- DVE
  C:  w_l = x_l + m*r_l               -- DVE mult + GPSIMD/DVE add
      x_l = conv5x5(w_l)              -- 5 accumulating fp32r matmuls with
                                         banded-Toeplitz weights g[dc]*B (row
                                         taps in the weights, col taps as
                                         shifted rhs windows), then one ACT
                                         PSUM->SBUF copy.

Bands are stored 516 wide with 2 zero-pad columns each side, so the col-tap
windows read zeros at image boundaries and every matmul dst is the full
[0,512) range (fp32r requires even dst start/size).
"""
import sys

sys.path.insert(0, "/opt/trn_rl_repo")
import numpy as np
import concourse.bass as bass
import concourse.mybir as mybir
import concourse.tile as tile
from concourse.bass_utils import run_bass_kernel_spmd

H, W, L = 512, 512, 28
N_ITER = 12
SIGMA = 0.5
PI = 3.141592653589793
NCORES = 8
ROWS = 128          # slab rows per core
OUT_ROWS = 64       # exact output rows per core
HALO = 32           # (ROWS - OUT_ROWS) / 2
WP = W + 4          # padded band pitch (2 zero cols each side)

f32 = mybir.dt.float32
f32r = mybir.dt.float32r


def _offsets(s, phi_deg):
    phi = phi_deg * PI / 180.0
    dx = s * np.cos(phi)
    dy = s * np.sin(phi)
    dx = dx - dx.min()
    dy = dy - dy.min()
    return np.rint(dx).astype(np.int32), np.rint(dy).astype(np.int32)


def _gauss1d(sigma):
    ksize = max(3, int(6 * sigma + 1) | 1)
    ax = np.arange(ksize, dtype=np.float32) - ksize // 2
    g1 = np.exp(-0.5 * (ax / sigma) ** 2)
    g1 = g1 / g1.sum()
    return g1.astype(np.float32)  # [5]


def _split_excess_waits(nc, max_w=1):
    """walrus in this toolchain accepts at most one sync wait per instruction;
    hoist excess waits onto preceding same-engine NoOp carriers."""
    ctr = 0
    for f in nc.m.functions:
        for bb in f.blocks:
            il = bb.instructions
            i = 0
            while i < len(il):
                inst = il[i]
                si = inst.sync_info
                w = list(si.on_wait) if (si and si.on_wait) else []
                if len(w) > max_w:
                    si.on_wait = w[-max_w:]
                    extra = w[:-max_w]
                    pos = i
                    for j in range(0, len(extra), max_w):
                        ctr += 1
                        nop = mybir.InstNoOp(
                            name=f"I-waitsplit-{ctr}", ins=[], outs=[]
                        )
                        nop.engine = inst.engine
                        nop.sync_info = mybir.SyncInfo(
                            on_wait=extra[j : j + max_w], on_update=[]
                        )
                        il.insert(pos, nop)
                        pos += 1
                        i += 1
                i += 1


def build_nc(dx, n_iter=N_ITER, w_add_engine="gpsimd"):
    """Build the SPMD Bass program. dx: tuple of L ints (column shifts)."""
    dx = [int(v) for v in dx]
    Wm = W + max(dx)   # measurement-plane width (539 nominal)
    EX = Wm - W        # 27
    EXe = EX + (EX % 2)  # even-padded scatter tail width (28)

    nc = bass.Bass()
    y_in = nc.declare_dram_parameter("y_slab", [ROWS, Wm], f32, isOutput=False)
    m_in = nc.declare_dram_parameter("m_slab", [ROWS, W], f32, isOutput=False)
    # weights: [I, g0*B, g1*B, g2*B, g3*B, g4*B] stacked -> [128, 6, 128]
    w_in = nc.declare_dram_parameter("wmats", [128, 6, 128], f32, isOutput=False)
    out = nc.declare_dram_parameter("xout", [L, OUT_ROWS, W], f32, isOutput=True)

    with tile.TileContext(nc) as tc:
        with (
            tc.tile_pool(name="state", bufs=1) as st,
            tc.tile_pool(name="ybps", bufs=2, space="PSUM") as ybp,
            tc.tile_pool(name="cps", bufs=3, space="PSUM") as cp,
        ):
            # ---- load inputs ----
            y_sb = st.tile([ROWS, Wm], f32)
            m_sb = st.tile([ROWS, W], f32)
            w32 = st.tile([128, 6, 128], f32)
            nc.sync.dma_start(y_sb[:], y_in[:])
            nc.sync.dma_start(m_sb[:], m_in[:])
            nc.sync.dma_start(w32[:], w_in[:])

            wr = st.tile([128, 6, 128], f32r)     # rounded weights
            nc.vector.tensor_copy(wr[:], w32[:])
            W_I = wr[:, 0, :]
            W_G = [wr[:, 1 + k, :] for k in range(5)]

            zf32 = st.tile([128, EXe], f32)
            nc.vector.memset(zf32[:], 0.0)
            zero_r = st.tile([128, EXe], f32r)
            nc.vector.tensor_copy(zero_r[:], zf32[:])

            # persistent padded tiles for the A-phase masked product u.
            # Even shifts write cols [0,512), odd shifts write [1,513); the
            # unwritten pad columns stay zero (separate buffer sets per
            # parity so the pads are never clobbered).
            NBUF = 3
            u_even = [st.tile([ROWS, 514], f32r, name=f"ue{i}") for i in range(NBUF)]
            u_odd = [st.tile([ROWS, 514], f32r, name=f"uo{i}") for i in range(NBUF)]
            # conv-input tiles w = x + m*r, padded like xs
            w_bufs = [st.tile([ROWS, WP], f32r, name=f"w{i}") for i in range(3)]
            zpad = st.tile([128, 2], f32)
            nc.vector.memset(zpad[:], 0.0)
            for t in u_odd:
                nc.vector.tensor_copy(t[:, 0:1], zpad[:, 0:1])
                nc.vector.tensor_copy(t[:, 512:514], zpad[:])
            for t in u_even:
                nc.vector.tensor_copy(t[:, 512:514], zpad[:])
            for t in w_bufs:
                nc.vector.tensor_copy(t[:, 0:2], zpad[:])
                nc.vector.tensor_copy(t[:, 514:516], zpad[:])

            # ---- Phi_sum = max(sum_l shift_l(m), 1);  invPhi = 1/Phi ----
            phi_sb = st.tile([ROWS, Wm], f32)
            phiB = st.tile([ROWS, Wm], f32)
            nc.vector.memset(phi_sb[:, W:], 0.0)
            nc.vector.memset(phiB[:], 0.0)
            nc.vector.tensor_copy(phi_sb[:, dx[0] : dx[0] + W], m_sb[:])
            nc.vector.tensor_copy(phiB[:, dx[1] : dx[1] + W], m_sb[:])
            for l in range(2, L):
                d = dx[l]
                tgt = phi_sb if l % 2 == 0 else phiB
                nc.vector.tensor_add(
                    out=tgt[:, d : d + W],
                    in0=tgt[:, d : d + W],
                    in1=m_sb[:],
                )
            nc.vector.tensor_add(out=phi_sb[:], in0=phi_sb[:], in1=phiB[:])
            nc.vector.tensor_scalar_max(phi_sb[:], phi_sb[:], 1.0)
            inv_phi = st.tile([ROWS, Wm], f32)
            nc.vector.reciprocal(inv_phi[:], phi_sb[:])

            # ---- x state [ROWS, L, WP], bands at cols [2, 514) ----
            xs = st.tile([ROWS, L, WP], f32r)
            nc.vector.tensor_copy(
                xs[:, :, 0:2], zpad[:, None, :].to_broadcast((ROWS, L, 2))
            )
            nc.vector.tensor_copy(
                xs[:, :, 514:516], zpad[:, None, :].to_broadcast((ROWS, L, 2))
            )
            for l in range(L):
                d = dx[l]
                eng = nc.gpsimd if l % 2 == 0 else nc.vector
                eng.tensor_mul(
                    out=xs[:, l, 2 : 2 + W], in0=m_sb[:], in1=y_sb[:, d : d + W]
                )
            mi_sb = st.tile([ROWS, L, W], f32)
            for l in range(L):
                d = dx[l]
                eng = nc.gpsimd if l % 2 == 1 else nc.vector
                eng.tensor_mul(
                    out=mi_sb[:, l, :], in0=m_sb[:], in1=inv_phi[:, d : d + W]
                )

            # ---- y1 init ----
            y1_sb = st.tile([ROWS, Wm], f32)
            nc.vector.tensor_copy(y1_sb[:], y_sb[:])
            r_sb = st.tile([ROWS, Wm], f32)
            t0_sb = st.tile([ROWS, Wm], f32)
            t1_sb = st.tile([ROWS, Wm], f32)

            w_add = nc.gpsimd if w_add_engine == "gpsimd" else nc.vector

            # ---- GAP iterations ----
            for it in range(n_iter):
                # phase A: yb = sum_l shift_l(m * x_l)
                yb = ybp.tile([ROWS, W + EXe], f32, tag="yb")
                nc.tensor.matmul(
                    yb[:, W : W + EXe], W_I, zero_r[:], start=True, stop=False,
                    skip_group_check=True,
                )
                n_even = 0
                n_odd = 0
                for l in range(L):
                    d = dx[l]
                    if d % 2 == 0:
                        u = u_even[n_even % NBUF]
                        n_even += 1
                    else:
                        u = u_odd[n_odd % NBUF]
                        n_odd += 1
                    off = d % 2  # odd shifts write at column offset 1
                    u_eng = nc.vector if l >= L - 10 else nc.gpsimd
                    u_eng.tensor_mul(
                        out=u[:, off : off + W], in0=m_sb[:], in1=xs[:, l, 2 : 2 + W]
                    )
                    if d % 2 == 0:
                        nc.tensor.matmul(
                            yb[:, d:W], W_I, u[:, : W - d],
                            start=(l == 0), stop=False, skip_group_check=True,
                        )
                        if d > 0:
                            nc.tensor.matmul(
                                yb[:, W : W + d], W_I, u[:, W - d : W],
                                start=False, stop=(l == L - 1),
                                skip_group_check=True,
                            )
                    else:
                        # u holds x*m at cols [1,513); u[0]=u[513]=0
                        nc.tensor.matmul(
                            yb[:, d - 1 : W], W_I, u[:, : W + 1 - d],
                            start=False, stop=False, skip_group_check=True,
                        )
                        nc.tensor.matmul(
                            yb[:, W : W + d + 1], W_I, u[:, W + 1 - d : 514],
                            start=False, stop=(l == L - 1),
                            skip_group_check=True,
                        )

                # phase B: t0 = y1 + y - 2*yb  (2-op chain; invPhi is folded
                # into the per-band masks mi).  y1 += y - yb off critical path.
                nc.vector.scalar_tensor_tensor(
                    out=t0_sb[:], in0=yb[:, :Wm], scalar=-2.0, in1=y1_sb[:],
                    op0=mybir.AluOpType.mult, op1=mybir.AluOpType.add,
                )
                nc.vector.scalar_tensor_tensor(
                    out=t0_sb[:], in0=t0_sb[:], scalar=1.0, in1=y_sb[:],
                    op0=mybir.AluOpType.mult, op1=mybir.AluOpType.add,
                )
                # phase C per band: x_l = conv5x5(x_l + m*r_l)
                for l in range(L):
                    d = dx[l]
                    w = w_bufs[l % 3]
                    nc.vector.tensor_mul(
                        out=w[:, 2 : 2 + W], in0=mi_sb[:, l, :], in1=t0_sb[:, d : d + W]
                    )
                    nc.vector.tensor_add(
                        out=w[:, 2 : 2 + W],
                        in0=w[:, 2 : 2 + W],
                        in1=xs[:, l, 2 : 2 + W],
                    )
                    x2 = cp.tile([ROWS, W], f32, tag="x2")
                    # out[:, j] += (g[dc]*B)^T w[:, j+dc+2], dc = -2..2
                    # center tap first so start=True covers the bank
                    for dc in (0, -2, -1, 1, 2):
                        nc.tensor.matmul(
                            x2[:, 0:W], W_G[dc + 2], w[:, dc + 2 : dc + 2 + W],
                            start=(dc == 0), stop=(dc == 2),
                            skip_group_check=True,
                        )
                    nc.scalar.copy(xs[:, l, 2 : 2 + W], x2[:])
                    if it == n_iter - 1:
                        nc.sync.dma_start(
                            out[l, :, :],
                            xs[HALO : HALO + OUT_ROWS, l, 2 : 2 + W].bitcast(f32),
                        )
                    if l == 26 and it < n_iter - 1:
                        # y1 += y - yb, deferred off the critical B->C path
                        nc.vector.scalar_tensor_tensor(
                            out=t1_sb[:], in0=yb[:, :Wm], scalar=-1.0,
                            in1=y_sb[:],
                            op0=mybir.AluOpType.mult, op1=mybir.AluOpType.add,
                        )
                        nc.vector.tensor_add(
                            out=y1_sb[:], in0=y1_sb[:], in1=t1_sb[:]
                        )


    _split_excess_waits(nc, max_w=1)
    return nc


def _host_inputs(y_1hw, mask2d, dx):
    """Per-core input maps."""
    y2 = np.asarray(y_1hw, dtype=np.float32)[0]      # [512, Wm]
    m2 = np.asarray(mask2d, dtype=np.float32)        # [512, 512]
    Wm = W + int(max(dx))
    g1 = _gauss1d(SIGMA)
    ident = np.eye(128, dtype=np.float32)

    in_maps = []
    for c in range(NCORES):
        rk = 64 * c - HALO
        y_slab = np.zeros((ROWS, Wm), dtype=np.float32)
        m_slab = np.zeros((ROWS, W), dtype=np.float32)
        lo = max(0, -rk)              # first valid slab row
        hi = min(ROWS, H - rk)        # one past last valid slab row
        y_slab[lo:hi] = y2[rk + lo : rk + hi]
        m_slab[lo:hi] = m2[rk + lo : rk + hi]
        # banded row-conv matrix, zeroed outside the valid (global) row range
        B = np.zeros((128, 128), dtype=np.float32)
        for k in range(-2, 3):
            for i in range(128):
                ip = i + k                      # input slab row
                if lo <= i < hi and lo <= ip < hi:
                    B[ip, i] = g1[k + 2]
        wm = np.zeros((128, 6, 128), dtype=np.float32)
        wm[:, 0, :] = ident
        for k in range(5):
            wm[:, 1 + k, :] = g1[k] * B
        in_maps.append({"y_slab": y_slab, "m_slab": m_slab, "wmats": wm})
    return in_maps


_NC_CACHE = {}


def _get_nc(dx, n_iter=N_ITER):
    key = (tuple(int(v) for v in dx), n_iter)
    if key not in _NC_CACHE:
        _NC_CACHE[key] = build_nc(key[0], n_iter)
    return _NC_CACHE[key]


def kernel(y_1hw, mask2d, phi_d_deg, s_nom, n_iter=N_ITER, trace=False):
    s = np.asarray(s_nom, dtype=np.float32)
    phi = float(np.asarray(phi_d_deg))
    dx, dy = _offsets(s, phi)
    assert (dy == 0).all(), "kernel assumes dy == 0 (row shifts unsupported)"
    nc = _get_nc(dx, n_iter)
    in_maps = _host_inputs(y_1hw, mask2d, dx)
    res = run_bass_kernel_spmd(
        nc, in_maps, list(range(NCORES)), trace=trace
    )
    x_full = np.empty((1, L, H, W), dtype=np.float32)
    for c in range(NCORES):
        x_full[0, :, 64 * c : 64 * (c + 1), :] = res.results[c]["xout"]
    kernel.last_results = res
    return x_full
